# Initial kernel scaffold
#
"""Baichuan paged-attention layer on 8 trn2 cores, tensor-parallel over heads.

Per core c: heads 4c..4c+3. Device computes QKV proj (f32r matmuls), RoPE,
attention vs [gathered history KV + new KV], and a partial o_proj
[T, HID] against w_o[:, 512c:512c+512]. Host gathers history KV pages,
builds RoPE/mask tables, and sums the 8 partial outputs.
"""
import sys

sys.path.insert(0, "/opt/trn_rl_repo")
import numpy as np

H = 32; D = 128; HID = 4096; BS = 64; NBLOCKS = 128
B = 4; QLEN = 512; MAXBLK = 24; ROPE_BASE = 10000.0
T = B * QLEN; NCORES = 8; HC = H // NCORES; W = HC * D  # 4 heads, 512 wide
NEG = -1.0e30
SCALE = 1.0 / float(np.sqrt(D))

_cache = {}
last_results = None  # BassKernelResults of the most recent run (for test.py)


def _round128(x):
    return (x + 127) // 128 * 128


def _build(hist):
    import concourse.bass as bass
    import concourse.tile as tile
    from concourse import bacc, mybir

    F32 = mybir.dt.float32
    F32R = mybir.dt.float32r

    def r(ap):
        return ap.bitcast(F32R)

    hv = [_round128(h) for h in hist]
    SH = [x // 128 for x in hv]

    nc = bacc.Bacc("TRN2", target_bir_lowering=False, debug=False,
                   num_devices=NCORES)
    hiddenT_d = nc.dram_tensor("hiddenT", [HID, T], F32, kind="ExternalInput")
    wqkvT_d = nc.dram_tensor("wqkvT", [HID, 3 * W], F32, kind="ExternalInput")
    woT_d = nc.dram_tensor("woT", [W, HID], F32, kind="ExternalInput")
    kh_d = [nc.dram_tensor(f"khT{b}", [W, hv[b]], F32, kind="ExternalInput")
            if hv[b] else None for b in range(B)]
    vh_d = [nc.dram_tensor(f"vh{b}", [hv[b], W], F32, kind="ExternalInput")
            if hv[b] else None for b in range(B)]
    out_d = nc.dram_tensor("out", [T, HID], F32, kind="ExternalOutput")

    # host-built tables baked into the NEFF
    inv = 1.0 / (ROPE_BASE ** (np.arange(0, D, 2) / D))
    pos = np.concatenate([h + np.arange(QLEN) for h in hist]).astype(np.float64)
    ang = np.concatenate([inv, inv])[:, None] * pos[None, :]
    cos_d = nc.inline_tensor(np.cos(ang).astype(np.float32), name="cosT")
    sin_d = nc.inline_tensor(np.sin(ang).astype(np.float32), name="sinT")

    mask_np = np.zeros((128, 4 * QLEN), np.float32)
    for j in range(4):
        srel = j * 128 + np.arange(128)[:, None]
        mask_np[:, j * QLEN:(j + 1) * QLEN] = np.where(
            srel <= np.arange(QLEN)[None, :], 0.0, NEG)
    mask_d = nc.inline_tensor(mask_np, name="mask4")

    pad_np = np.zeros((128, B), np.float32)
    for b in range(B):
        if hv[b]:
            pad_np[:, b] = np.where(hv[b] - 128 + np.arange(128) >= hist[b],
                                    NEG, 0.0)
    pad_d = nc.inline_tensor(pad_np, name="padc")

    Pm = np.zeros((128, 128), np.float32)
    for d in range(64):
        Pm[d, d + 64] = -1.0
        Pm[d + 64, d] = 1.0
    pt_d = nc.inline_tensor(np.ascontiguousarray(Pm.T), name="permT")
    ones_d = nc.inline_tensor(np.ones((128, 1), np.float32), name="ones")
    onesr_d = nc.inline_tensor(np.ones((1, 128), np.float32), name="onesr")

    with tile.TileContext(nc) as tc:
        with tc.tile_pool(name="const", bufs=1) as cpool, \
             tc.tile_pool(name="attn", bufs=16) as apool, \
             tc.tile_pool(name="psum", bufs=8, space="PSUM") as pspool:
            mask_t = cpool.tile([128, 4 * QLEN], F32, tag="mask")
            nc.sync.dma_start(mask_t[:], mask_d[:])
            pad_t = cpool.tile([128, B], F32, tag="pad")
            nc.sync.dma_start(pad_t[:], pad_d[:])
            pt_t = cpool.tile([128, 128], F32, tag="pt")
            nc.sync.dma_start(pt_t[:], pt_d[:])
            ones_t = cpool.tile([128, 1], F32, tag="ones")
            nc.sync.dma_start(ones_t[:], ones_d[:])
            onesr_t = cpool.tile([1, 128], F32, tag="onesr")
            nc.sync.dma_start(onesr_t[:], onesr_d[:])

            attn_sb = [[None] * HC for _ in range(B)]

            with tc.tile_pool(name="cs", bufs=2) as cspool, \
                 tc.tile_pool(name="hid", bufs=32) as hidpool, \
                 tc.tile_pool(name="wst", bufs=8) as wqpool, \
                 tc.tile_pool(name="wvst", bufs=4) as wvpool, \
                 tc.tile_pool(name="qkr", bufs=8) as qkrpool, \
                 tc.tile_pool(name="rope", bufs=2) as rppool, \
                 tc.tile_pool(name="vsb", bufs=4) as vpool, \
                 tc.tile_pool(name="khp", bufs=2) as khpool, \
                 tc.tile_pool(name="vhp", bufs=8) as vhpool, \
                 tc.tile_pool(name="expp", bufs=3) as epool, \
                 tc.tile_pool(name="smol", bufs=2) as smpool:
                for b in range(B):
                    tsl = slice(b * QLEN, (b + 1) * QLEN)
                    cos_t = cspool.tile([128, QLEN], F32, tag="cos")
                    nc.sync.dma_start(cos_t[:], cos_d[:, tsl])
                    sin_t = cspool.tile([128, QLEN], F32, tag="sin")
                    nc.sync.dma_start(sin_t[:], sin_d[:, tsl])

                    hid_t = []
                    for k in range(32):
                        ht = hidpool.tile([128, QLEN], F32, tag="hid")
                        nc.sync.dma_start(
                            ht[:], hiddenT_d[k * 128:(k + 1) * 128, tsl])
                        hid_t.append(ht)

                    # ---- QK projection + RoPE: qk_rot[rt] = [128 d, 512 t]
                    qk_rot = []
                    for rt in range(8):
                        pq = pspool.tile([128, QLEN], F32, tag="ps")
                        for k in range(32):
                            wt = wqpool.tile([128, 128], F32, tag="wq")
                            nc.sync.dma_start(
                                wt[:], wqkvT_d[k * 128:(k + 1) * 128,
                                               rt * 128:(rt + 1) * 128])
                            nc.tensor.matmul(pq[:], r(wt[:]), r(hid_t[k][:]),
                                             start=(k == 0), stop=(k == 31))
                        qs = rppool.tile([128, QLEN], F32, tag="qs")
                        nc.scalar.copy(qs[:], pq[:])
                        rot = pspool.tile([128, QLEN], F32, tag="ps")
                        nc.tensor.matmul(rot[:], r(pt_t[:]), r(qs[:]),
                                         start=True, stop=True)
                        t1 = rppool.tile([128, QLEN], F32, tag="t1")
                        nc.vector.tensor_mul(t1[:], rot[:], sin_t[:])
                        t2 = rppool.tile([128, QLEN], F32, tag="t2")
                        nc.vector.tensor_mul(t2[:], qs[:], cos_t[:])
                        qr = qkrpool.tile([128, QLEN], F32, tag="qkr")
                        nc.vector.tensor_add(qr[:], t1[:], t2[:])
                        qk_rot.append(qr)

                    # ---- V projection: v_sb[tt] = [128 t, 512 hd]
                    v_sb = []
                    for tt in range(4):
                        v_sb.append(vpool.tile([128, W], F32, tag="vsb"))
                    for k in range(32):
                        wv = wvpool.tile([128, W], F32, tag="wv")
                        nc.sync.dma_start(
                            wv[:], wqkvT_d[k * 128:(k + 1) * 128, 2 * W:3 * W])
                        for tt in range(4):
                            if k == 0:
                                pv0 = pspool.tile([128, W], F32, tag="ps")
                                v_ps[tt] = pv0  # noqa: F821
                            nc.tensor.matmul(
                                v_ps[tt][:],  # noqa: F821
                                r(hid_t[k][:, tt * 128:(tt + 1) * 128]),
                                r(wv[:]), start=(k == 0), stop=(k == 31))
                    for tt in range(4):
                        nc.vector.tensor_copy(v_sb[tt][:], v_ps[tt][:])  # noqa: F821

                    # ---- history KV loads
                    vh_t = []
                    for st in range(SH[b]):
                        vt = vhpool.tile([128, W], F32, tag="vh")
                        nc.sync.dma_start(
                            vt[:], vh_d[b][st * 128:(st + 1) * 128, :])
                        vh_t.append(vt)

                    # ---- attention per head
                    S = SH[b] + 4
                    for h in range(HC):
                        kh_t = None
                        if SH[b]:
                            kh_t = khpool.tile([128, hv[b]], F32, tag="kh")
                            nc.sync.dma_start(
                                kh_t[:], kh_d[b][h * 128:(h + 1) * 128, :])
                        dn = pspool.tile([1, QLEN], F32, tag="ps")
                        pv = pspool.tile([128, QLEN], F32, tag="ps")
                        for st in range(S):
                            sc = pspool.tile([128, QLEN], F32, tag="ps")
                            if st < SH[b]:
                                lhsT = kh_t[:, st * 128:(st + 1) * 128]
                            else:
                                j = st - SH[b]
                                lhsT = qk_rot[4 + h][:, j * 128:(j + 1) * 128]
                            nc.tensor.matmul(sc[:], r(lhsT), r(qk_rot[h][:]),
                                             start=True, stop=True)
                            if st == SH[b] - 1 and hist[b] != hv[b]:
                                nc.vector.tensor_scalar_add(
                                    sc[:], sc[:], pad_t[:, b:b + 1])
                            if st >= SH[b]:
                                j = st - SH[b]
                                nc.vector.tensor_add(
                                    sc[:], sc[:],
                                    mask_t[:, j * QLEN:(j + 1) * QLEN])
                            ex = epool.tile([128, QLEN], F32, tag="exp")
                            nc.scalar.activation(
                                ex[:], sc[:], mybir.ActivationFunctionType.Exp,
                                scale=SCALE)
                            nc.tensor.matmul(dn[:], r(ones_t[:]), r(ex[:]),
                                             start=(st == 0), stop=(st == S - 1))
                            if st < SH[b]:
                                vt = vh_t[st][:, h * 128:(h + 1) * 128]
                            else:
                                vt = v_sb[st - SH[b]][:, h * 128:(h + 1) * 128]
                            nc.tensor.matmul(pv[:], r(vt), r(ex[:]),
                                             start=(st == 0), stop=(st == S - 1))
                        rc = smpool.tile([1, QLEN], F32, tag="rc")
                        nc.vector.reciprocal(rc[:], dn[:])
                        bc = pspool.tile([128, QLEN], F32, tag="ps")
                        nc.tensor.matmul(bc[:], r(onesr_t[:]), r(rc[:]),
                                         start=True, stop=True)
                        bcs = smpool.tile([128, QLEN], F32, tag="bcs")
                        nc.vector.tensor_copy(bcs[:], bc[:])
                        at = apool.tile([128, QLEN], F32, tag="attn")
                        nc.vector.tensor_mul(at[:], pv[:], bcs[:])
                        attn_sb[b][h] = at

            # ---- o_proj partial: out[t, i] += attn^T . woT
            with tc.tile_pool(name="wop", bufs=8) as wopool, \
                 tc.tile_pool(name="stg", bufs=8) as stpool:
                for ic in range(8):
                    isl = slice(ic * 512, (ic + 1) * 512)
                    wo_t = []
                    for jt in range(4):
                        wt = wopool.tile([128, 512], F32, tag="wo")
                        nc.sync.dma_start(
                            wt[:], woT_d[jt * 128:(jt + 1) * 128, isl])
                        wo_t.append(wt)
                    for tt in range(16):
                        b, q = tt // 4, tt % 4
                        po = pspool.tile([128, 512], F32, tag="ps")
                        for jt in range(4):
                            nc.tensor.matmul(
                                po[:],
                                r(attn_sb[b][jt][:, q * 128:(q + 1) * 128]),
                                r(wo_t[jt][:]), start=(jt == 0), stop=(jt == 3))
                        st_ = stpool.tile([128, 512], F32, tag="stg")
                        if tt % 2 == 0:
                            nc.vector.tensor_copy(st_[:], po[:])
                        else:
                            nc.scalar.copy(st_[:], po[:])
                        nc.sync.dma_start(
                            out_d[tt * 128:(tt + 1) * 128, isl], st_[:])
    nc.compile()
    return {"nc": nc}


def _get(hist):
    if hist not in _cache:
        _cache[hist] = _build(hist)
    return _cache[hist]


def kernel(**inputs):
    global last_results
    from concourse.bass_utils import run_bass_kernel_spmd

    hidden = np.asarray(inputs["hidden_states"], np.float32)
    w_pack = np.asarray(inputs["w_pack"], np.float32)
    w_o = np.asarray(inputs["w_o"], np.float32)
    kc = np.asarray(inputs["key_cache"], np.float32).reshape(NBLOCKS * BS, H, D)
    vc = np.asarray(inputs["value_cache"], np.float32).reshape(NBLOCKS * BS, H, D)
    bo = np.asarray(inputs["block_offsets"], np.int32)
    hist = tuple(int(x) for x in np.asarray(inputs["history_lengths"]))
    assert all(0 <= h and h + QLEN <= MAXBLK * BS for h in hist)
    hv = [_round128(h) for h in hist]

    built = _get(hist)
    hiddenT = np.ascontiguousarray(hidden.T)

    in_maps = []
    for c in range(NCORES):
        rs = slice(c * W, (c + 1) * W)
        wqkv = np.concatenate(
            [w_pack[rs], w_pack[HID + c * W:HID + (c + 1) * W],
             w_pack[2 * HID + c * W:2 * HID + (c + 1) * W]], axis=0)
        im = {
            "hiddenT": hiddenT,
            "wqkvT": np.ascontiguousarray(wqkv.T),
            "woT": np.ascontiguousarray(w_o[:, rs].T),
        }
        for b in range(B):
            if not hv[b]:
                continue
            nblk = (hist[b] + BS - 1) // BS
            rows = (bo[b, :nblk, None] * BS +
                    np.arange(BS)[None, :]).reshape(-1)[:hist[b]]
            khp = np.zeros((hv[b], HC, D), np.float32)
            khp[:hist[b]] = kc[rows][:, c * HC:(c + 1) * HC, :]
            vhp = np.zeros((hv[b], HC, D), np.float32)
            vhp[:hist[b]] = vc[rows][:, c * HC:(c + 1) * HC, :]
            im[f"khT{b}"] = np.ascontiguousarray(
                khp.transpose(1, 2, 0).reshape(W, hv[b]))
            im[f"vh{b}"] = np.ascontiguousarray(vhp.reshape(hv[b], W))
        in_maps.append(im)

    last_results = run_bass_kernel_spmd(built["nc"], in_maps,
                                        core_ids=list(range(NCORES)))
    acc = np.zeros((T, HID), np.float64)
    for c in range(NCORES):
        acc += last_results.results[c]["out"]
    return acc.astype(np.float32)


# revision 7
# speedup vs baseline: 1.0478x; 1.0478x over previous
"""Baichuan paged-attention layer on 8 trn2 cores, tensor-parallel over heads.

Per core c: heads 4c..4c+3. Device computes QKV proj (f32r matmuls), RoPE,
attention vs [gathered history KV + new KV], and a partial o_proj
[T, HID] against w_o[:, 512c:512c+512]. Host gathers history KV pages,
builds RoPE/mask tables, and sums the 8 partial outputs.
"""
import sys

sys.path.insert(0, "/opt/trn_rl_repo")
import numpy as np

H = 32; D = 128; HID = 4096; BS = 64; NBLOCKS = 128
B = 4; QLEN = 512; MAXBLK = 24; ROPE_BASE = 10000.0
T = B * QLEN; NCORES = 8; HC = H // NCORES; W = HC * D  # 4 heads, 512 wide
NEG = -1.0e30
SCALE = 1.0 / float(np.sqrt(D))

_cache = {}
last_results = None  # BassKernelResults of the most recent run (for test.py)


def _round128(x):
    return (x + 127) // 128 * 128


def _build(hist):
    import concourse.bass as bass
    import concourse.tile as tile
    from concourse import bacc, mybir

    F32 = mybir.dt.float32
    F32R = mybir.dt.float32r

    hv = [_round128(h) for h in hist]
    SH = [x // 128 for x in hv]

    nc = bacc.Bacc("TRN2", target_bir_lowering=False, debug=False,
                   num_devices=NCORES)
    hiddenT_d = nc.dram_tensor("hiddenT", [HID, T], F32R, kind="ExternalInput")
    wqkvT_d = nc.dram_tensor("wqkvT", [HID, 3 * W], F32R, kind="ExternalInput")
    woT_d = nc.dram_tensor("woT", [W, HID], F32R, kind="ExternalInput")
    kh_d = [nc.dram_tensor(f"khT{b}", [W, hv[b]], F32R, kind="ExternalInput")
            if hv[b] else None for b in range(B)]
    vh_d = [nc.dram_tensor(f"vh{b}", [hv[b], W], F32R, kind="ExternalInput")
            if hv[b] else None for b in range(B)]
    out_d = nc.dram_tensor("out", [T, HID], F32, kind="ExternalOutput")

    # host-built tables baked into the NEFF
    inv = 1.0 / (ROPE_BASE ** (np.arange(0, D, 2) / D))
    pos = np.concatenate([h + np.arange(QLEN) for h in hist]).astype(np.float64)
    ang = np.concatenate([inv, inv])[:, None] * pos[None, :]
    cos_d = nc.inline_tensor(np.cos(ang).astype(np.float32), name="cosT")
    sin_d = nc.inline_tensor(np.sin(ang).astype(np.float32), name="sinT")

    mask_np = np.zeros((128, 4 * QLEN), np.float32)
    for j in range(4):
        srel = j * 128 + np.arange(128)[:, None]
        mask_np[:, j * QLEN:(j + 1) * QLEN] = np.where(
            srel <= np.arange(QLEN)[None, :], 0.0, NEG)
    mask_d = nc.inline_tensor(mask_np, name="mask4")

    pad_np = np.zeros((128, B), np.float32)
    for b in range(B):
        if hv[b]:
            pad_np[:, b] = np.where(hv[b] - 128 + np.arange(128) >= hist[b],
                                    NEG, 0.0)
    pad_d = nc.inline_tensor(pad_np, name="padc")

    Pm = np.zeros((128, 128), np.float32)
    for d in range(64):
        Pm[d, d + 64] = -1.0
        Pm[d + 64, d] = 1.0
    pt_d = nc.inline_tensor(np.ascontiguousarray(Pm.T), name="permT")
    ones_d = nc.inline_tensor(np.ones((128, 1), np.float32), name="ones")
    onesr_d = nc.inline_tensor(np.ones((1, 128), np.float32), name="onesr")

    with tile.TileContext(nc) as tc:
        with tc.tile_pool(name="const", bufs=1) as cpool, \
             tc.tile_pool(name="attn", bufs=16) as apool, \
             tc.tile_pool(name="psum", bufs=8, space="PSUM") as pspool:
            mask_t = cpool.tile([128, 4 * QLEN], F32, tag="mask")
            nc.sync.dma_start(mask_t[:], mask_d[:])
            pad_t = cpool.tile([128, B], F32, tag="pad")
            nc.sync.dma_start(pad_t[:], pad_d[:])
            pt_t = cpool.tile([128, 128], F32R, tag="pt")
            nc.sync.dma_start(pt_t[:], pt_d[:].bitcast(F32R))
            ones_t = cpool.tile([128, 1], F32R, tag="ones")
            nc.sync.dma_start(ones_t[:], ones_d[:].bitcast(F32R))
            onesr_t = cpool.tile([1, 128], F32, tag="onesr")
            nc.sync.dma_start(onesr_t[:], onesr_d[:])

            attn_sb = [[None] * HC for _ in range(B)]

            with tc.tile_pool(name="cs", bufs=2) as cspool, \
                 tc.tile_pool(name="hid", bufs=32) as hidpool, \
                 tc.tile_pool(name="wst", bufs=8) as wqpool, \
                 tc.tile_pool(name="wvst", bufs=4) as wvpool, \
                 tc.tile_pool(name="qkr", bufs=8) as qkrpool, \
                 tc.tile_pool(name="rope", bufs=2) as rppool, \
                 tc.tile_pool(name="vsb", bufs=4) as vpool, \
                 tc.tile_pool(name="khp", bufs=2) as khpool, \
                 tc.tile_pool(name="vhp", bufs=8) as vhpool, \
                 tc.tile_pool(name="expp", bufs=3) as epool, \
                 tc.tile_pool(name="smol", bufs=2) as smpool:
                for b in range(B):
                    tsl = slice(b * QLEN, (b + 1) * QLEN)
                    cos_t = cspool.tile([128, QLEN], F32, tag="cos")
                    nc.sync.dma_start(cos_t[:], cos_d[:, tsl])
                    sin_t = cspool.tile([128, QLEN], F32, tag="sin")
                    nc.sync.dma_start(sin_t[:], sin_d[:, tsl])

                    hid_t = []
                    for k in range(32):
                        ht = hidpool.tile([128, QLEN], F32R, tag="hid")
                        nc.sync.dma_start(
                            ht[:], hiddenT_d[k * 128:(k + 1) * 128, tsl])
                        hid_t.append(ht)

                    # ---- QK projection + RoPE: qk_rot[rt] = [128 d, 512 t]
                    qk_rot = []
                    for rt in range(8):
                        pq = pspool.tile([128, QLEN], F32, tag="ps")
                        for k in range(32):
                            wt = wqpool.tile([128, 128], F32R, tag="wq")
                            nc.sync.dma_start(
                                wt[:], wqkvT_d[k * 128:(k + 1) * 128,
                                               rt * 128:(rt + 1) * 128])
                            nc.tensor.matmul(pq[:], wt[:], hid_t[k][:],
                                             start=(k == 0), stop=(k == 31))
                        qs = rppool.tile([128, QLEN], F32R, tag="qs")
                        nc.scalar.copy(qs[:], pq[:])
                        rot = pspool.tile([128, QLEN], F32, tag="ps")
                        nc.tensor.matmul(rot[:], pt_t[:], qs[:],
                                         start=True, stop=True)
                        t1 = rppool.tile([128, QLEN], F32, tag="t1")
                        nc.vector.tensor_mul(t1[:], rot[:], sin_t[:])
                        t2 = rppool.tile([128, QLEN], F32, tag="t2")
                        nc.vector.tensor_mul(t2[:], qs[:], cos_t[:])
                        qr = qkrpool.tile([128, QLEN], F32R, tag="qkr")
                        nc.vector.tensor_add(qr[:], t1[:], t2[:])
                        qk_rot.append(qr)

                    # ---- V projection: v_sb[tt] = [128 t, 512 hd]
                    v_sb = [vpool.tile([128, W], F32R, tag="vsb", name=f"vsb{b}_{i}")
                            for i in range(4)]
                    v_ps = [pspool.tile([128, W], F32, tag="ps", name=f"vps{b}_{i}")
                            for i in range(4)]
                    for k in range(32):
                        wv = wvpool.tile([128, W], F32R, tag="wv")
                        nc.sync.dma_start(
                            wv[:], wqkvT_d[k * 128:(k + 1) * 128, 2 * W:3 * W])
                        for tt in range(4):
                            nc.tensor.matmul(
                                v_ps[tt][:],
                                hid_t[k][:, tt * 128:(tt + 1) * 128],
                                wv[:], start=(k == 0), stop=(k == 31))
                    for tt in range(4):
                        nc.vector.tensor_copy(v_sb[tt][:], v_ps[tt][:])

                    # ---- history KV loads
                    vh_t = []
                    for st in range(SH[b]):
                        vt = vhpool.tile([128, W], F32R, tag="vh")
                        nc.sync.dma_start(
                            vt[:], vh_d[b][st * 128:(st + 1) * 128, :])
                        vh_t.append(vt)

                    # ---- attention per head
                    S = SH[b] + 4
                    for h in range(HC):
                        kh_t = None
                        if SH[b]:
                            kh_t = khpool.tile([128, hv[b]], F32R, tag="kh")
                            nc.sync.dma_start(
                                kh_t[:], kh_d[b][h * 128:(h + 1) * 128, :])
                        dn = pspool.tile([1, QLEN], F32, tag="ps")
                        pv = pspool.tile([128, QLEN], F32, tag="ps")
                        for st in range(S):
                            sc = pspool.tile([128, QLEN], F32, tag="ps")
                            if st < SH[b]:
                                lhsT = kh_t[:, st * 128:(st + 1) * 128]
                            else:
                                j = st - SH[b]
                                lhsT = qk_rot[4 + h][:, j * 128:(j + 1) * 128]
                            nc.tensor.matmul(sc[:], lhsT, qk_rot[h][:],
                                             start=True, stop=True)
                            if st == SH[b] - 1 and hist[b] != hv[b]:
                                nc.vector.tensor_scalar_add(
                                    sc[:], sc[:], pad_t[:, b:b + 1])
                            if st >= SH[b]:
                                j = st - SH[b]
                                nc.vector.tensor_add(
                                    sc[:], sc[:],
                                    mask_t[:, j * QLEN:(j + 1) * QLEN])
                            ex = epool.tile([128, QLEN], F32R, tag="exp")
                            nc.scalar.activation(
                                ex[:], sc[:], mybir.ActivationFunctionType.Exp,
                                scale=SCALE)
                            nc.tensor.matmul(dn[:], ones_t[:], ex[:],
                                             start=(st == 0), stop=(st == S - 1))
                            if st < SH[b]:
                                vt = vh_t[st][:, h * 128:(h + 1) * 128]
                            else:
                                vt = v_sb[st - SH[b]][:, h * 128:(h + 1) * 128]
                            nc.tensor.matmul(pv[:], vt, ex[:],
                                             start=(st == 0), stop=(st == S - 1))
                        rc = smpool.tile([1, QLEN], F32, tag="rc")
                        nc.vector.reciprocal(rc[:], dn[:])
                        bc = pspool.tile([128, QLEN], F32, tag="ps")
                        nc.tensor.matmul(bc[:], onesr_t[:], rc[:],
                                         start=True, stop=True)
                        bcs = smpool.tile([128, QLEN], F32, tag="bcs")
                        nc.vector.tensor_copy(bcs[:], bc[:])
                        at = apool.tile([128, QLEN], F32R, tag="attn")
                        nc.vector.tensor_mul(at[:], pv[:], bcs[:])
                        attn_sb[b][h] = at

            # ---- o_proj partial: out[t, i] += attn^T . woT
            with tc.tile_pool(name="wop", bufs=8) as wopool, \
                 tc.tile_pool(name="stg", bufs=8) as stpool:
                for ic in range(8):
                    isl = slice(ic * 512, (ic + 1) * 512)
                    wo_t = []
                    for jt in range(4):
                        wt = wopool.tile([128, 512], F32R, tag="wo")
                        nc.sync.dma_start(
                            wt[:], woT_d[jt * 128:(jt + 1) * 128, isl])
                        wo_t.append(wt)
                    for tt in range(16):
                        b, q = tt // 4, tt % 4
                        po = pspool.tile([128, 512], F32, tag="ps")
                        for jt in range(4):
                            nc.tensor.matmul(
                                po[:],
                                attn_sb[b][jt][:, q * 128:(q + 1) * 128],
                                wo_t[jt][:], start=(jt == 0), stop=(jt == 3))
                        st_ = stpool.tile([128, 512], F32, tag="stg")
                        if tt % 2 == 0:
                            nc.vector.tensor_copy(st_[:], po[:])
                        else:
                            nc.scalar.copy(st_[:], po[:])
                        nc.sync.dma_start(
                            out_d[tt * 128:(tt + 1) * 128, isl], st_[:])
    nc.compile()
    return {"nc": nc}


def _get(hist):
    if hist not in _cache:
        _cache[hist] = _build(hist)
    return _cache[hist]


def prepare_in_maps(inputs):
    hidden = np.asarray(inputs["hidden_states"], np.float32)
    w_pack = np.asarray(inputs["w_pack"], np.float32)
    w_o = np.asarray(inputs["w_o"], np.float32)
    kc = np.asarray(inputs["key_cache"], np.float32).reshape(NBLOCKS * BS, H, D)
    vc = np.asarray(inputs["value_cache"], np.float32).reshape(NBLOCKS * BS, H, D)
    bo = np.asarray(inputs["block_offsets"], np.int32)
    hist = tuple(int(x) for x in np.asarray(inputs["history_lengths"]))
    assert all(0 <= h and h + QLEN <= MAXBLK * BS for h in hist)
    hv = [_round128(h) for h in hist]

    built = _get(hist)
    hiddenT = np.ascontiguousarray(hidden.T)

    in_maps = []
    for c in range(NCORES):
        rs = slice(c * W, (c + 1) * W)
        wqkv = np.concatenate(
            [w_pack[rs], w_pack[HID + c * W:HID + (c + 1) * W],
             w_pack[2 * HID + c * W:2 * HID + (c + 1) * W]], axis=0)
        im = {
            "hiddenT": hiddenT,
            "wqkvT": np.ascontiguousarray(wqkv.T),
            "woT": np.ascontiguousarray(w_o[:, rs].T),
        }
        for b in range(B):
            if not hv[b]:
                continue
            nblk = (hist[b] + BS - 1) // BS
            rows = (bo[b, :nblk, None] * BS +
                    np.arange(BS)[None, :]).reshape(-1)[:hist[b]]
            khp = np.zeros((hv[b], HC, D), np.float32)
            khp[:hist[b]] = kc[rows][:, c * HC:(c + 1) * HC, :]
            vhp = np.zeros((hv[b], HC, D), np.float32)
            vhp[:hist[b]] = vc[rows][:, c * HC:(c + 1) * HC, :]
            im[f"khT{b}"] = np.ascontiguousarray(
                khp.transpose(1, 2, 0).reshape(W, hv[b]))
            im[f"vh{b}"] = np.ascontiguousarray(vhp.reshape(hv[b], W))
        in_maps.append(im)

    return built["nc"], in_maps


def kernel(**inputs):
    global last_results
    from concourse.bass_utils import run_bass_kernel_spmd

    nc, in_maps = prepare_in_maps(inputs)
    last_results = run_bass_kernel_spmd(nc, in_maps,
                                        core_ids=list(range(NCORES)))
    acc = np.zeros((T, HID), np.float64)
    for c in range(NCORES):
        acc += last_results.results[c]["out"]
    return acc.astype(np.float32)
